# revision 32
# baseline (speedup 1.0000x reference)
"""Trainium2 Bass kernel for the vq_codebook CCE loss.

Live dataflow of the reference:
    t   = (1/(B*F)) * sum_b min_p ||outputs[b] - clusters[tc_b, p]||^2
    out = ALPHA*t + BETA*(1 - t)
Only the TARGET class's prototype distances feed the loss (the wrong-class
branch of the reference is dead code), so per batch row only 32 of the
6400 prototype distances are live.

Strategy (8 NeuronCores, SPMD):
  - Host sorts rows by target class (stable) and splits the sorted batch
    into 16 tiles of 128 rows.  Each tile's rows span a small contiguous
    class range (<=16 classes for random data), so a single 512-column
    PSUM bank holds every prototype column any of its rows needs.
  - Each core takes 2 tiles.  Per tile: 3 fp8 DoubleRow matmuls (256
    contraction rows each) compute -2*x.c for the gathered columns, then
    one rank-20 DoubleRow matmul adds ||c||^2 (split 16*h + r, fp8,
    abs err <= 2) AND a +224*224 shift on every column outside the row's
    own class window (the one-hot select mask is rank-16, so it rides the
    same matmul; the shift cancels exactly on the row's own window).
  - A single full-row DVE min per tile then yields each row's selected
    nearest-prototype distance directly - no mask/select stage.
  - ||x||^2 comes from one Scalar-engine Square pass with accum_out over
    the core's fp8 x slice (a = -2x, so sum a^2 = 4 sum x^2).
  - A final f32 ones-matmul reduces [128,3] partials across partitions to
    [1,3] so the output DMA is a single descriptor.
  - Host combines: t = (sum x2 + sum selected_min)/(B*F).

fp8 e4m3 quantization moves t by ~0.03% (validated off-device vs f64).
NOTE: tensor_tensor_reduce crashes the exec unit on this HW (bisected);
do not reintroduce it.
"""

import os
import numpy as np
import ml_dtypes  # noqa: F401  (np dtype registry for bf16/fp8)
from contextlib import ExitStack

import concourse.tile as tile
from concourse import bacc, mybir
from concourse.bass_utils import run_bass_kernel_spmd

ALPHA = 5.0
BETA = 5.0

B, F, C, P = 2048, 768, 200, 32
NCORES = 8
NT = B // 128            # 16 row tiles of 128 sorted rows
TPC = NT // NCORES       # 2 tiles per core
K3 = F // 256            # 3 DoubleRow contraction chunks
RPC = 128 * TPC          # 256 rows per core
BIG = 224.0              # BIG*BIG = 50176 shift for non-selected columns

F32 = mybir.dt.float32
BF16 = mybir.dt.bfloat16
KDT = mybir.dt.float8e4
AX = mybir.AxisListType
OP = mybir.AluOpType

V_DMA = os.environ.get("KV_DMA", "3")  # cg stream pieces: 1 | 3
V_WU = int(os.environ.get("KV_WU", "0"))   # PE warm-up dummy matmuls
V_OUT = os.environ.get("KV_OUT", "direct")  # direct | pe

_prog_cache = {}


def _build_program(nb):
    """nb = PSUM banks per tile (1 unless some tile spans >16 classes)."""
    key = ("nc", nb, V_DMA, V_WU, V_OUT)
    if key in _prog_cache:
        return _prog_cache[key]

    ncol = 512 * nb          # prototype columns per tile
    mbr = TPC * 2 * ncol     # rhs part of mb
    mbl = TPC * nb * 256     # lhsT part of mb

    nc = bacc.Bacc(
        "TRN2", target_bir_lowering=False, debug=False, num_devices=NCORES,
        enable_asserts=False, enable_partition_id=False,
    )

    xa = nc.dram_tensor("xa", [128, K3 * 2 * RPC], KDT, kind="ExternalInput").ap()
    cg = nc.dram_tensor("cg", [128, K3 * TPC * 2 * ncol], KDT, kind="ExternalInput").ap()
    mb = nc.dram_tensor("mb", [10, mbr + mbl], KDT, kind="ExternalInput").ap()
    out = nc.dram_tensor(
        "out", [1, 3] if V_OUT == "pe" else [128, 3], F32, kind="ExternalOutput"
    ).ap()

    DR = mybir.MatmulPerfMode.DoubleRow

    with tile.TileContext(nc) as tc, ExitStack() as ctx:
        const = ctx.enter_context(tc.tile_pool(name="const", bufs=1))
        psum = ctx.enter_context(tc.tile_pool(name="psum", bufs=2 * nb, space="PSUM"))
        psco = ctx.enter_context(tc.tile_pool(name="psco", bufs=1, space="PSUM"))

        xa_sb = const.tile([128, K3 * 2 * RPC], KDT, name="xa_sb", tag="xa")
        cg_sb = const.tile([128, K3 * TPC * 2 * ncol], KDT, name="cg_sb", tag="cg")
        mb_sb = const.tile([10, mbr + mbl], KDT, name="mb_sb", tag="mb")
        sq = const.tile([128, K3 * 2 * RPC], BF16, name="sq", tag="sq")
        res = const.tile([128, 3], F32, name="res", tag="res")
        ones = const.tile([128, 1], F32, name="ones", tag="on")
        outs = const.tile([1, 3], F32, name="outs", tag="os")

        xa_v = xa_sb[:].rearrange("p (k s r) -> p k s r", k=K3, s=2)
        cg_v = cg_sb[:].rearrange("p (k t s j) -> p k t s j", k=K3, t=TPC, s=2)
        mbr_v = mb_sb[:, 0:mbr].rearrange("p (t s j) -> p t s j", t=TPC, s=2)
        mbl_v = mb_sb[:, mbr:].rearrange("p (t b s r) -> p t b s r", t=TPC, b=nb, s=2)

        # --- PE warm-up: dummy matmuls release the HAM clock gate (PE runs
        # at 1.2 GHz until ~3.4us of sustained activity) while the DMAs
        # stream, so the real matmuls run at 2.4 GHz. ---
        if V_WU:
            dum = const.tile([128, 512], BF16, name="dum", tag="dum")
            onesb = const.tile([128, 1], BF16, name="onesb", tag="ob")
            psd = psco.tile([1, 512], F32, name="psd", tag="psd")
            nc.gpsimd.memset(dum[:], 0.0)
            nc.gpsimd.memset(onesb[:], 1.0)
            for _ in range(V_WU):
                nc.tensor.matmul(
                    psd[:], lhsT=onesb[:], rhs=dum[:], start=True, stop=True
                )

        # --- DMAs: no dep chains; xa then cg stream on the sync HWDGE ring
        # in PE-consumption order, mb rides the scalar HWDGE ring. ---
        nc.sync.dma_start(xa_sb[:], xa)
        if V_DMA == "1":
            nc.sync.dma_start(cg_sb[:], cg)
        else:
            cg_f = cg_sb[:].rearrange("p (k x) -> p k x", k=K3)
            cg_d = cg.rearrange("p (k x) -> p k x", k=K3)
            for k in range(K3):
                nc.sync.dma_start(cg_f[:, k, :], cg_d[:, k, :])
        nc.scalar.dma_start(mb_sb[:], mb)

        if V_OUT == "pe":
            nc.gpsimd.memset(ones[:], 1.0)

        # --- sum x^2 on the Scalar engine in the DMA/PE shadow ---
        nc.scalar.activation(
            out=sq[:], in_=xa_sb[:],
            func=mybir.ActivationFunctionType.Square,
            accum_out=res[:, 2:3],
        )

        # --- PE: per tile, 2 DoubleRow chunks, the rank-20 c2+select
        # matmul (needs only mb), then the last chunk with stop=True so
        # the DVE min fires the moment it retires. ---
        pss = [psum.tile([128, ncol], F32, name="ps", tag="ps") for _ in range(TPC)]

        def mm_chunk(t, k, start, stop):
            for b in range(nb):
                nc.tensor.matmul(
                    pss[t][:, b * 512 : (b + 1) * 512],
                    lhsT=xa_v[:, k, :, t * 128 : (t + 1) * 128],
                    rhs=cg_v[:, k, t, :, b * 512 : (b + 1) * 512],
                    perf_mode=DR,
                    start=start,
                    stop=stop,
                )

        for k in range(K3 - 1):
            for t in range(TPC):
                mm_chunk(t, k, start=(k == 0), stop=False)
        for t in range(TPC):
            for b in range(nb):
                nc.tensor.matmul(
                    pss[t][:, b * 512 : (b + 1) * 512],
                    lhsT=mbl_v[:, t, b, :, :],
                    rhs=mbr_v[:, t, :, b * 512 : (b + 1) * 512],
                    perf_mode=DR,
                    start=False,
                    stop=False,
                )
        for t in range(TPC):
            mm_chunk(t, K3 - 1, start=False, stop=True)

        # --- DVE: one full-row min per tile = the selected distance ---
        for t in range(TPC):
            nc.vector.tensor_reduce(
                out=res[:, t : t + 1],
                in_=pss[t][:],
                axis=AX.X,
                op=OP.min,
            )

        if V_OUT == "pe":
            # cross-partition reduce on the PE, then a 1-descriptor DMA
            pco = psco.tile([1, 3], F32, name="pco", tag="pco")
            nc.tensor.matmul(pco[:], lhsT=ones[:], rhs=res[:], start=True, stop=True)
            nc.scalar.copy(out=outs[:], in_=pco[:])
            nc.sync.dma_start(out, outs[:])
        else:
            nc.sync.dma_start(out, res[:])

    nc.compile()
    _prog_cache[key] = nc
    return nc


def _prep_inputs(outputs, clusters, target_classes):
    outputs = np.ascontiguousarray(np.asarray(outputs, dtype=np.float32))
    clusters = np.ascontiguousarray(np.asarray(clusters, dtype=np.float32))
    tc_np = np.asarray(target_classes).astype(np.int64)

    np_k = mybir.dt.np(KDT)

    order = np.argsort(tc_np, kind="stable")
    xs = outputs[order]
    tcs = tc_np[order]

    los = np.empty(NT, np.int64)
    spans = np.empty(NT, np.int64)
    for t in range(NT):
        seg = tcs[t * 128 : (t + 1) * 128]
        los[t] = seg.min()
        spans[t] = seg.max() - seg.min() + 1
    nb = max(1, int(-(-int(spans.max()) // 16)))
    ncol = 512 * nb
    nw = ncol // 32
    mbr = TPC * 2 * ncol
    mbl = TPC * nb * 256

    flat = clusters.reshape(C * P, F)
    c2 = (flat.astype(np.float64) ** 2).sum(axis=1).astype(np.float32)

    # -2x in fp8, laid out (p, k, s, r): feature = k*256 + s*128 + p
    a8 = np.clip(-2.0 * xs, -240, 240).astype(np_k)  # [B, F]

    big8 = np.float32(BIG).astype(np_k)

    in_maps = []
    for ci in range(NCORES):
        rows = slice(ci * RPC, (ci + 1) * RPC)
        xa_i = np.ascontiguousarray(
            a8[rows].T.reshape(K3, 2, 128, RPC).transpose(2, 0, 1, 3)
            .reshape(128, K3 * 2 * RPC)
        )

        cg_i = np.zeros((128, K3, TPC, 2, ncol), np_k)
        mb_i = np.zeros((10, mbr + mbl), np_k)
        mbr_v = mb_i[:, 0:mbr].reshape(10, TPC, 2, ncol)
        mbl_v = mb_i[:, mbr:].reshape(10, TPC, nb, 2, 128)
        for tt in range(TPC):
            t = ci * TPC + tt
            lo = int(los[t])
            hi = min(lo + nw, C)
            npro = (hi - lo) * P
            G = flat[lo * P : hi * P]                       # [npro, F]
            g8 = np.clip(G, -240, 240).astype(np_k)
            # (F, npro) -> (k, s, p, npro) -> (p, k, s, npro)
            cg_i[:, :, tt, :, :npro] = (
                g8.T.reshape(K3, 2, 128, npro).transpose(2, 0, 1, 3)
            )
            c2t = np.zeros(ncol, np.float32)
            c2t[:npro] = c2[lo * P : hi * P]
            h8 = np.clip(c2t / 16.0, -240, 240).astype(np_k)
            r8 = np.clip(c2t - 16.0 * h8.astype(np.float32), -240, 240).astype(np_k)
            # rhs components (partition kk, slot s) = comp 2*kk+s:
            #   0: h, 1: r, 2: +BIG const, 3+w: -BIG on window w's columns
            mbr_v[0, tt, 0, :] = h8
            mbr_v[0, tt, 1, :] = r8
            mbr_v[1, tt, 0, :] = big8
            wincol = np.repeat(np.arange(nw), P)            # window of each col
            for w in range(nw):
                comp = 3 + (w % 16)                         # bank-local component
                mbr_v[comp // 2, tt, comp % 2, wincol == w] = -big8
            # lhsT components: 0: 16, 1: 1, 2: BIG, 3+w: BIG iff row's
            # window == w (per bank: component 3+wl maps window b*16+wl)
            w_r = (tcs[t * 128 : (t + 1) * 128] - lo).astype(np.int64)
            mbl_v[0, tt, :, 0, :] = np.float32(16.0).astype(np_k)
            mbl_v[0, tt, :, 1, :] = np.float32(1.0).astype(np_k)
            mbl_v[1, tt, :, 0, :] = big8
            for bk in range(nb):
                for wl in range(16):
                    comp = 3 + wl
                    sel = w_r == bk * 16 + wl
                    mbl_v[comp // 2, tt, bk, comp % 2, sel] = big8

        in_maps.append(
            {
                "xa": xa_i,
                "cg": np.ascontiguousarray(cg_i.reshape(128, -1)),
                "mb": np.ascontiguousarray(mb_i),
            }
        )
    return nb, in_maps


def _finish(results):
    s = 0.0
    for r in results:
        o = r["out"].astype(np.float64)
        s += o[:, 0].sum() + o[:, 1].sum() + o[:, 2].sum() / 4.0
    t = np.float32(s / (B * F))
    ans = np.float32(ALPHA) * t + np.float32(BETA) * (np.float32(1.0) - t)
    return np.asarray(ans, dtype=np.float32)


def kernel(outputs, clusters, target_classes, _run_kwargs=None):
    nb, in_maps = _prep_inputs(outputs, clusters, target_classes)
    nc = _build_program(nb)
    kw = _run_kwargs or {}
    res = run_bass_kernel_spmd(nc, in_maps, list(range(NCORES)), **kw)
    ans = _finish(res.results)
    if _run_kwargs is not None:
        kernel.last_result = res
    return ans


if __name__ == "__main__":
    rng = np.random.default_rng(0)
    o = rng.standard_normal((B, F), dtype=np.float32)
    cl = rng.standard_normal((C, P, F), dtype=np.float32)
    t = rng.integers(0, C, size=(B,)).astype(np.int32)
    print(kernel(o, cl, t))
